# revision 1
# baseline (speedup 1.0000x reference)
"""TRN2 Bass kernel for nn_Adapter (dense_cnn): ViT adapter with two branches
  main:   h1 = xs@w1+b1 ; y = dwconv3d_3x3x3(h1)+cb ; y@w2+b2
  offset: g = xs@ow1    ; hoff = tdiff(g)+ob1 ; oc = dwconv_1x3x3(hoff)+ocb ; oc@ow2+ob2
  out = x with patch tokens += main + offset   (CLS rows pass through)

Data-parallel over 8 NeuronCores: 2 clips (16 frames) per core; adapter
weights replicated. Per-core kernel (raw bass, explicit semaphores,
fine-grained pipelining, fp8-e4m3 DoubleRow matmuls):
  - activations channel-major; x^T supplied pre-transposed (fp8) per shard
  - all three matmul stages run fp8 DoubleRow (2 contraction rows/cycle);
    depthwise convs are PSUM-accumulated diagonal DR matmuls on PE with
    taps paired within equal-dw groups (DR mid-step must be 16-aligned)
  - every weight stage is scaled x16 to keep fp8 values out of e4m3
    subnormals; the final evict multiplies by 1/16^3 (fused, free)
  - a tunable subset of conv chunks runs on DVE (bf16 scratch accumulator,
    final cast to fp8) to balance engines; matmul2 is token-major and the
    DVE evict fuses scale + the f32 +x residual in one op
  - m2 phase is paired (2-tile out-DMAs/evicts/prefetched xtok) so the SP
    sequencer and DMA bandwidth stay off the critical loop; all cross-DMA
    semaphore waits are fanout-safe (dedicated or rotating sem pools)

Self-contained: hardcodes shapes for x:[128,197,768], T=8 (asserts).
"""
import numpy as np
import ml_dtypes

import concourse.bass as bass
import concourse.mybir as mybir
from concourse.bass_utils import run_bass_kernel_spmd

F32 = mybir.dt.float32
BF16 = mybir.dt.bfloat16
F8 = mybir.dt.float8e4
AOT = mybir.AluOpType
AFT = mybir.ActivationFunctionType
DR = mybir.MatmulPerfMode.DoubleRow
BF = ml_dtypes.bfloat16
F8NP = ml_dtypes.float8_e4m3

# ---- problem constants (per core) ----
C = 768
CA = 384
T = 8
NPL = 256
CLIPS = 2
NPIX_CLIP = T * 14 * 14
NPIX = CLIPS * NPIX_CLIP
KC = C // 128
NG = CA // 128
H1PAD = NG * CLIPS * (T + 2) * NPL
GPAD = NG * CLIPS * T * NPL
GUARD = NPL
NTIL2 = (NPIX + 127) // 128
M1_CH = 392
OUT_ROWS = NPIX + 16
CVS = 16.0   # weight up-scale per stage; /CVS**3 folded into final evict

# tap (dt, dh, dw) lists grouped by dw so DR pairs share dw (step % 16 == 0)
def _pairs(taps):
    by_dw = {}
    for tp in taps:
        by_dw.setdefault(tp[2], []).append(tp)
    prs = []
    for dw in sorted(by_dw):
        grp = by_dw[dw]
        for i in range(0, len(grp) - 1, 2):
            prs.append((grp[i], grp[i + 1]))
        if len(grp) % 2:
            prs.append((grp[-1], None))
    return prs

MAIN_TAPS = [(kd - 1, kh - 1, kw - 1)
             for kd in range(3) for kh in range(3) for kw in range(3)]
OFF_TAPS = [(0, kh - 1, kw - 1) for kh in range(3) for kw in range(3)]
MAIN_PAIRS = _pairs(MAIN_TAPS)   # 15 (12 pairs + 3 singles)
OFF_PAIRS = _pairs(OFF_TAPS)     # 6 (3 pairs + 3 singles)
NPR_MAIN = len(MAIN_PAIRS)
NPR_OFF = len(OFF_PAIRS)
NPR_TOT = (NPR_MAIN + NPR_OFF) * NG   # 63

CONV_CHUNKS = [(br, g, c, tc)
               for c in range(CLIPS) for tc in range(4)
               for br in (0, 1) for g in range(NG)]


def _dve_set(n_main, n_off):
    mains = [ch for ch in CONV_CHUNKS if ch[0] == 1]
    offs = [ch for ch in CONV_CHUNKS if ch[0] == 0]
    return set(mains[:n_main]) | set(offs[:n_off])


DVE_CHUNKS = _dve_set(4, 0)


def build(debug=False, dve_chunks=None):
    dvec = DVE_CHUNKS if dve_chunks is None else dve_chunks
    nc = bass.Bass()
    xT = nc.declare_dram_parameter("xT", [C, NPIX], F8, isOutput=False)
    xtok = nc.declare_dram_parameter("xtok", [NPIX, C], F32, isOutput=False)
    xcls = nc.declare_dram_parameter("xcls", [16, C], F32, isOutput=False)
    w1c = nc.declare_dram_parameter("w1c", [128, KC // 2 * 2 * C], F8, isOutput=False)
    w2c = nc.declare_dram_parameter("w2c", [128, KC // 2 * 2 * C], F8, isOutput=False)
    diag = nc.declare_dram_parameter("diag", [128, NPR_TOT * 2 * 128], F8, isOutput=False)
    b1c = nc.declare_dram_parameter("b1c", [128, KC], F32, isOutput=False)
    cbc = nc.declare_dram_parameter("cbc", [128, KC], F32, isOutput=False)
    wtp = nc.declare_dram_parameter("wtp", [128, (27 + 9) * NG], F32, isOutput=False)
    zeros = nc.declare_dram_parameter("zeros", [1, 3584], F8, isOutput=False)
    out = nc.declare_dram_parameter("out", [OUT_ROWS, C], F32, isOutput=True)
    if debug:
        dbg_h1 = nc.declare_dram_parameter("dbg_h1", [128, H1PAD + 2 * GUARD], F8, isOutput=True)
        dbg_g = nc.declare_dram_parameter("dbg_g", [128, GPAD + 2 * GUARD], F8, isOutput=True)
        dbg_cv = nc.declare_dram_parameter("dbg_cv", [128, KC * NPIX], F8, isOutput=True)

    xT_sb = nc.alloc_sbuf_tensor([128, KC * NPIX], F8)
    w1_sb = nc.alloc_sbuf_tensor([128, KC // 2 * 2 * C], F8)   # [pair][s][m]
    w2_sb = nc.alloc_sbuf_tensor([128, KC // 2 * 2 * C], F8)
    diag_sb = nc.alloc_sbuf_tensor([128, NPR_TOT * 2 * 128], F8)  # [pr][s][m]
    b1_sb = nc.alloc_sbuf_tensor([128, KC], F32)
    cb_sb = nc.alloc_sbuf_tensor([128, KC], F32)
    wt_sb = nc.alloc_sbuf_tensor([128, (27 + 9) * NG], F32)
    h1p = nc.alloc_sbuf_tensor([128, H1PAD + 2 * GUARD], F8)
    gp = nc.alloc_sbuf_tensor([128, GPAD + 2 * GUARD], F8)
    cvo = nc.alloc_sbuf_tensor([128, KC * NPIX], F8)
    scr = nc.alloc_sbuf_tensor([128, 2 * 196], BF16)    # DVE conv scratch (2 bufs)
    zsb = nc.alloc_sbuf_tensor([128, 960], F8)          # zeroed tile for halo fills
    xtk = nc.alloc_sbuf_tensor([128, NTIL2 * C], F32)
    ost = nc.alloc_sbuf_tensor([128, 8 * C], F32)
    ps = nc.alloc_psum_tensor([128, 4096], F32)

    M_ORDER = [3, 4, 5, 0, 1, 2]

    def h1_plane(g, c, tpad):
        return GUARD + ((g * CLIPS + c) * (T + 2) + tpad) * NPL

    def g_plane(g, c, t):
        return GUARD + ((g * CLIPS + c) * T + t) * NPL

    def sv3(buf, ext, offset, dims):
        """3D free view [part + dims] of an sbuf tensor via explicit AP."""
        return bass.AP(buf, offset, [[ext, 128]] + [list(d) for d in dims])

    XT_EXT = KC * NPIX
    W_EXT = KC // 2 * 2 * C
    DG_EXT = NPR_TOT * 2 * 128
    H1_EXT = H1PAD + 2 * GUARD
    GP_EXT = GPAD + 2 * GUARD
    CV_EXT = KC * NPIX

    # ---------- static schedules ----------
    m1_chunks = [(mi, m, j) for mi, m in enumerate(M_ORDER) for j in range(8)]
    conv_pe = [ch for ch in CONV_CHUNKS if ch not in dvec]
    conv_dve = [ch for ch in CONV_CHUNKS if ch in dvec]
    N_M1 = len(m1_chunks)
    N_CPE = len(conv_pe)
    N_CDVE = len(conv_dve)
    ACT_ALL = 2 * (N_M1 + N_CPE)
    MS_GP, MS_ALL = 4, 8                 # DVE halo-memset incs
    DIFF_ALL = MS_ALL + NG * CLIPS
    DVE_CONV_DONE = DIFF_ALL + N_CDVE
    DVE_EVP = lambda p: DVE_CONV_DONE + p + 1    # evict-pair p (p=12: final single)
    # s_ld carries b1+cb+wtp (wait LD_WALL = all three, fanout-safe)
    LD_WALL = 48

    def m1_thr(br, g, c, tc):
        mi = (3 + g) if br else g
        jmax = c * 4 + min(tc + 1, 3) if br else c * 4 + 3
        return 2 * (mi * 8 + jmax) + 2

    def conv_counts_thru(c_hi, tc_hi):
        npe = ndve = 0
        for ch in CONV_CHUNKS:
            br, g, c, tc = ch
            if (c, tc) > (c_hi, tc_hi):
                continue
            if ch in dvec:
                ndve += 1
            else:
                npe += 1
        return npe, ndve

    M2_THR = []
    for i in range(NTIL2):
        p_hi = (min(128 * (i + 1), NPIX) - 1) // 196
        c_hi, t_hi = divmod(p_hi, T)
        npe, ndve = conv_counts_thru(c_hi, t_hi // 2)
        M2_THR.append((2 * (N_M1 + npe), DIFF_ALL + ndve))

    from contextlib import ExitStack
    _sems = ExitStack()
    xk = [_sems.enter_context(nc.semaphore(f"s_xk{i}")) for i in range(8)]
    ot = [_sems.enter_context(nc.semaphore(f"s_ot{i}")) for i in range(8)]
    with (
        _sems,
        nc.Block() as block,
        nc.semaphore("s_ld") as s_ld,
        nc.semaphore("s_w1") as s_w1,
        nc.semaphore("s_xt0") as s_xt0,
        nc.semaphore("s_xt1") as s_xt1,
        nc.semaphore("s_xt2") as s_xt2,
        nc.semaphore("s_xt3") as s_xt3,
        nc.semaphore("s_dg") as s_dg,
        nc.semaphore("s_w2") as s_w2,
        nc.semaphore("s_z1") as s_z1,
        nc.semaphore("s_z2") as s_z2,
        nc.semaphore("s_pe") as s_pe,
        nc.semaphore("s_act") as s_act,
        nc.semaphore("s_dve") as s_dve,
        nc.semaphore("s_out") as s_out,
        nc.semaphore("s_dbg") as s_dbg,
    ):
        # ================= SP: all DMA =================
        @block.sync
        def _(sync):
            sync.dma_start(out=w1_sb[:], in_=w1c[:]).then_inc(s_w1, 16)
            for qq, sx in ((0, s_xt0), (1, s_xt1)):
                sync.dma_start(
                    out=xT_sb[:].rearrange("p (k n) -> p k n", k=KC)[:, :, qq * 784:(qq + 1) * 784],
                    in_=xT[:].rearrange("(k p) n -> p k n", p=128)[:, :, qq * 784:(qq + 1) * 784],
                ).then_inc(sx, 16)
            # zero-fill: gp guards (2 runs) + h1p t-halo/guard runs (7x512)
            sync.dma_start(
                out=bass.AP(gp, 0, [[GP_EXT, 128], [GUARD + GPAD, 2], [1, GUARD]]),
                in_=bass.AP(zeros, 0, [[0, 128], [GUARD, 2], [1, GUARD]]),
            ).then_inc(s_z1, 16)
            sync.dma_start(out=zsb[:],
                           in_=bass.AP(zeros, 0, [[0, 128], [1, 960]])).then_inc(s_z1, 16)
            sync.dma_start(out=b1_sb[:], in_=b1c[:]).then_inc(s_ld, 16)
            sync.dma_start(out=cb_sb[:], in_=cbc[:]).then_inc(s_ld, 16)
            sync.dma_start(out=wt_sb[:], in_=wtp[:]).then_inc(s_ld, 16)
            sync.dma_start(
                out=bass.AP(h1p, 0, [[H1_EXT, 128], [2560, 7], [1, 512]]),
                in_=bass.AP(zeros, 0, [[0, 128], [512, 7], [1, 512]]),
            ).then_inc(s_z2, 16)
            for qq, sx in ((2, s_xt2), (3, s_xt3)):
                sync.dma_start(
                    out=xT_sb[:].rearrange("p (k n) -> p k n", k=KC)[:, :, qq * 784:(qq + 1) * 784],
                    in_=xT[:].rearrange("(k p) n -> p k n", p=128)[:, :, qq * 784:(qq + 1) * 784],
                ).then_inc(sx, 16)
            sync.dma_start(out=w2_sb[:], in_=w2c[:]).then_inc(s_w2, 16)
            sync.dma_start(out=diag_sb[:], in_=diag[:]).then_inc(s_dg, 16)
            sync.dma_start(out=out[NPIX:OUT_ROWS, :], in_=xcls[:]).then_inc(s_out, 16)
            for pj in range(12):     # all xtok pair-loads up front
                j = 2 * pj
                sync.dma_start(
                    out=xtk[:, j * C:(j + 2) * C].rearrange("p (b c) -> p b c", b=2),
                    in_=xtok[j * 128:(j + 2) * 128, :].rearrange("(b r) c -> r b c", b=2),
                ).then_inc(xk[pj % 8], 16)
            sync.dma_start(out=xtk[:64, bass.ts(24, C)],
                           in_=xtok[24 * 128:NPIX, :]).then_inc(xk[12 % 8], 16)
            if debug:
                sync.wait_ge(s_act, 2 * N_M1)
                sync.wait_ge(s_dve, DIFF_ALL)
                sync.dma_start(out=dbg_h1[:], in_=h1p[:]).then_inc(s_dbg, 16)
                sync.dma_start(out=dbg_g[:], in_=gp[:]).then_inc(s_dbg, 16)
                sync.wait_ge(s_act, ACT_ALL)
                sync.wait_ge(s_dve, DVE_CONV_DONE)
                sync.dma_start(out=dbg_cv[:], in_=cvo[:]).then_inc(s_dbg, 16)
            for p in range(NTIL2 // 2):          # 12 pairs
                i = 2 * p
                sync.wait_ge(s_dve, DVE_EVP(p))
                sync.dma_start(
                    out=out[i * 128:(i + 2) * 128, :].rearrange("(b r) c -> r b c", b=2),
                    in_=ost[:, (p % 4) * 2 * C:((p % 4) * 2 + 2) * C].rearrange("p (b c) -> p b c", b=2),
                ).then_inc(ot[p % 8], 16)
            sync.wait_ge(s_dve, DVE_EVP(12))
            sync.dma_start(out=out[24 * 128:NPIX, :],
                           in_=ost[:64, bass.ts((12 % 4) * 2, C)]).then_inc(ot[12 % 8], 16)
            if debug:
                sync.wait_ge(s_dbg, 48)

        # ================= PE =================
        @block.tensor
        def _(tensor):
            tensor.wait_ge(s_w1, 16)
            # ---- matmul1 (DR, 3 chunk-pairs), banks 0..7 rotating ----
            for q, (mi, m, j) in enumerate(m1_chunks):
                bank = q % 8
                if q >= 8:
                    tensor.wait_ge(s_act, 2 * (q - 8) + 2)
                pv = ps[:, bank * 512: bank * 512 + M1_CH]
                if q in (0, 2, 4, 6):
                    tensor.wait_ge((s_xt0, s_xt1, s_xt2, s_xt3)[q // 2], 16)
                for pr in range(KC // 2):
                    lhsT = sv3(w1_sb, W_EXT, pr * 2 * C + m * 128,
                               [(C, 2), (1, 128)])
                    rhs = sv3(xT_sb, XT_EXT, (pr * 2) * NPIX + j * M1_CH,
                              [(NPIX, 2), (1, M1_CH)])
                    mm = tensor.matmul(pv, lhsT, rhs, perf_mode=DR,
                                       start=(pr == 0), stop=(pr == KC // 2 - 1))
                mm.then_inc(s_pe, 1)
            # ---- conv (PE chunks, DR pairs), banks 4..7 rotating ----
            tensor.wait_ge(s_dg, 16)
            for qc, (br, g, c, tc) in enumerate(conv_pe):
                bank = 4 + qc % 4
                if qc >= 4:
                    tensor.wait_ge(s_act, 2 * (N_M1 + qc - 4) + 2)
                else:
                    tensor.wait_ge(s_act, 2 * (44 + qc) + 2)
                tensor.wait_ge(s_act, m1_thr(br, g, c, tc))
                if br == 0:
                    tensor.wait_ge(s_dve, MS_ALL + g * 2 + c + 1)
                pairs = MAIN_PAIRS if br else OFF_PAIRS
                prbase = 0 if br else NPR_MAIN * NG
                pv = ps[:, bank * 512:(bank + 1) * 512]
                for ip, (tA, tB) in enumerate(pairs):
                    dtA, dhA, dwA = tA
                    if br:
                        offA = h1_plane(g, c, 2 * tc + 1 + dtA) + dhA * 16 + dwA
                        buf, ext = h1p, H1_EXT
                    else:
                        offA = g_plane(g, c, 2 * tc + dtA) + dhA * 16 + dwA
                        buf, ext = gp, GP_EXT
                    if tB is None:
                        sstep = 16
                    else:
                        dtB, dhB, dwB = tB
                        sstep = (dtB - dtA) * 256 + (dhB - dhA) * 16
                    lhsT = sv3(diag_sb, DG_EXT, (prbase + ip * NG + g) * 256,
                               [(128, 2), (1, 128)])
                    rhs = sv3(buf, ext, offA, [(sstep, 2), (1, 512)])
                    mm = tensor.matmul(pv, lhsT, rhs, perf_mode=DR,
                                       start=(ip == 0), stop=(ip == len(pairs) - 1),
                                       skip_group_check=True)
                mm.then_inc(s_pe, 1)
            # ---- matmul2 (DR, group-pairs), psum pairs {0,1}/{2,3} ----
            tensor.wait_ge(s_w2, 16)
            for i in range(NTIL2):
                rows = min(128, NPIX - i * 128)
                ta, td = M2_THR[i]
                tensor.wait_ge(s_act, ta)
                tensor.wait_ge(s_dve, td)
                if i == 2:
                    tensor.wait_ge(s_act, ACT_ALL)   # banks 4..7 freed by conv
                if i >= 4:
                    tensor.wait_ge(s_dve, DVE_EVP((i - 4) // 2))
                pv = ps[:rows, (i % 4) * 1024:(i % 4) * 1024 + 768]
                for pr in range(KC // 2):
                    lhsT = sv3(cvo, CV_EXT, (pr * 2) * NPIX + i * 128,
                               [(NPIX, 2), (1, rows)])
                    tensor.matmul(pv[:, 0:512], lhsT,
                                  sv3(w2_sb, W_EXT, pr * 2 * C, [(C, 2), (1, 512)]),
                                  perf_mode=DR,
                                  start=(pr == 0), stop=(pr == KC // 2 - 1),
                                  skip_group_check=True)
                    mm1 = tensor.matmul(pv[:, 512:768], lhsT,
                                        sv3(w2_sb, W_EXT, pr * 2 * C + 512,
                                            [(C, 2), (1, 256)]),
                                        perf_mode=DR,
                                        start=(pr == 0), stop=(pr == KC // 2 - 1),
                                        skip_group_check=True)
                mm1.then_inc(s_pe, 1)

        # ================= ACT: psum evicts =================
        @block.scalar
        def _(scalar):
            scalar.wait_ge(s_ld, LD_WALL)
            h1v = h1p[:, GUARD:GUARD + H1PAD].rearrange(
                "p (qq h w) -> p qq h w", h=16, w=16)
            gv = gp[:, GUARD:GUARD + GPAD].rearrange(
                "p (qq h w) -> p qq h w", h=16, w=16)
            seen_h1 = False
            for q, (mi, m, j) in enumerate(m1_chunks):
                bank = q % 8
                scalar.wait_ge(s_pe, q + 1)
                if q == 0:
                    scalar.wait_ge(s_z1, 32)
                    scalar.wait_ge(s_dve, MS_GP)
                if m < 3 and not seen_h1:
                    scalar.wait_ge(s_z2, 16)
                    scalar.wait_ge(s_dve, MS_ALL)
                    seen_h1 = True
                for pl in range(2):
                    gt = 2 * j + pl
                    c, t = divmod(gt, T)
                    src = ps[:, bank * 512 + pl * 196: bank * 512 + (pl + 1) * 196
                             ].rearrange("p (h w) -> p h w", h=14)
                    if m < 3:
                        dst = h1v[:, (m * CLIPS + c) * (T + 2) + t + 1, 1:15, 1:15]
                        bias = b1_sb[:, m:m + 1]
                    else:
                        dst = gv[:, ((m - 3) * CLIPS + c) * T + t, 1:15, 1:15]
                        bias = 0.0
                    scalar.activation(dst, src, AFT.Identity,
                                      bias=bias).then_inc(s_act, 1)
            for qc, (br, g, c, tc) in enumerate(conv_pe):
                bank = 4 + qc % 4
                scalar.wait_ge(s_pe, N_M1 + qc + 1)
                grp = g if br else 3 + g
                for pl in range(2):
                    t = 2 * tc + pl
                    src = ps[:, bank * 512 + pl * NPL + 17:
                             bank * 512 + pl * NPL + 17 + 14 * 16
                             ].rearrange("p (h w) -> p h w", w=16)[:, :, 0:14]
                    dst = cvo[:, grp * NPIX + c * NPIX_CLIP + t * 196:
                              grp * NPIX + c * NPIX_CLIP + (t + 1) * 196
                              ].rearrange("p (h w) -> p h w", h=14)
                    scalar.activation(dst, src, AFT.Identity,
                                      bias=cb_sb[:, grp:grp + 1]).then_inc(s_act, 1)

        # ================= DVE =================
        @block.vector
        def _(vector):
            # halo zero-fills: rows 0/15 and cols 0/15 of every padded plane
            # (tensor_copy from a DMA-zeroed tile; DVE memset is unreliable)
            vector.wait_ge(s_z1, 32)       # gp guards + zsb
            for buf, ext, npl_ in ((gp, GP_EXT, 48), (h1p, H1_EXT, 60)):
                for off, dims in (
                    (GUARD, [[256, npl_], [1, 16]]),           # row 0
                    (GUARD + 240, [[256, npl_], [1, 16]]),     # row 15
                    (GUARD, [[256, npl_], [16, 16]]),          # col 0
                    (GUARD + 15, [[256, npl_], [16, 16]]),     # col 15
                ):
                    vector.tensor_copy(
                        bass.AP(buf, off, [[ext, 128]] + dims),
                        bass.AP(zsb, 0, [[960, 128], [16, npl_], [1, 16]]),
                    ).then_inc(s_dve, 1)
            vector.wait_ge(s_ld, LD_WALL)
            for g in range(NG):
                for c in range(CLIPS):
                    vector.wait_ge(s_act, 2 * (g * 8 + c * 4 + 4))
                    for t in range(T - 1, 0, -1):
                        a = g_plane(g, c, t)
                        b = g_plane(g, c, t - 1)
                        last = vector.tensor_tensor(
                            gp[:, a:a + NPL], gp[:, a:a + NPL], gp[:, b:b + NPL],
                            op=AOT.subtract)
                    z = g_plane(g, c, 0)
                    last = vector.tensor_tensor(
                        gp[:, z:z + NPL], gp[:, z:z + NPL], gp[:, z:z + NPL],
                        op=AOT.subtract)
                    ob1 = b1_sb[:, 3 + g:4 + g]
                    for t in range(T):
                        base = g_plane(g, c, t)
                        iv = gp[:, base + 17: base + 17 + 14 * 16].rearrange(
                            "p (h w) -> p h w", w=16)[:, :, 0:14]
                        last = vector.tensor_scalar(iv, iv, ob1, None, op0=AOT.add)
                    last.then_inc(s_dve, 1)
            # ---- conv chunks on DVE (bf16 scratch acc, cast to fp8 at end) ----
            for br, g, c, tc in conv_dve:
                if br == 1:
                    vector.wait_ge(s_act, m1_thr(br, g, c, tc))
                taps = MAIN_TAPS if br else OFF_TAPS
                grp = g if br else 3 + g
                wbase = (0 if br else 27 * NG)
                for pl in range(2):
                    t = 2 * tc + pl
                    acc = scr[:, pl * 196:(pl + 1) * 196].rearrange(
                        "p (h w) -> p h w", h=14)
                    for it, (dt, dh, dw) in enumerate(taps):
                        if br:
                            base = h1_plane(g, c, t + 1 + dt)
                            srcbuf = h1p
                        else:
                            base = g_plane(g, c, t + dt)
                            srcbuf = gp
                        svv = srcbuf[:, base + 17 + dh * 16 + dw:
                                     base + 17 + dh * 16 + dw + 14 * 16
                                     ].rearrange("p (h w) -> p h w", w=16)[:, :, 0:14]
                        wsc = wt_sb[:, wbase + it * NG + g: wbase + it * NG + g + 1]
                        if it == 0:
                            vector.tensor_scalar(
                                acc, svv, wsc, cb_sb[:, grp:grp + 1],
                                op0=AOT.mult, op1=AOT.add)
                        else:
                            vector.scalar_tensor_tensor(
                                acc, svv, wsc, acc, op0=AOT.mult, op1=AOT.add)
                    dst = cvo[:, grp * NPIX + c * NPIX_CLIP + t * 196:
                              grp * NPIX + c * NPIX_CLIP + (t + 1) * 196
                              ].rearrange("p (h w) -> p h w", h=14)
                    last = vector.tensor_copy(dst, acc)
                last.then_inc(s_dve, 1)
            # ---- m2 evict + residual (paired: 2 tiles per op) ----
            for p in range(NTIL2 // 2 + 1):
                i = 2 * p
                if p == 12:      # final single tile, 64 rows
                    vector.wait_ge(s_pe, N_M1 + N_CPE + 25)
                    vector.wait_ge(xk[12 % 8], 16 * (12 // 8 + 1))
                    jj = (24 - 4) // 2
                    vector.wait_ge(ot[jj % 8], 16 * (jj // 8 + 1))
                    vector.scalar_tensor_tensor(
                        ost[:64, bass.ts((12 % 4) * 2, C)],
                        ps[:64, (24 % 4) * 1024:(24 % 4) * 1024 + 768],
                        1.0 / (CVS ** 3),
                        xtk[:64, bass.ts(24, C)],
                        op0=AOT.mult, op1=AOT.add).then_inc(s_dve, 1)
                    break
                vector.wait_ge(s_pe, N_M1 + N_CPE + i + 2)
                vector.wait_ge(xk[p % 8], 16 * (p // 8 + 1))
                if p >= 4:
                    jj = p - 4
                    vector.wait_ge(ot[jj % 8], 16 * (jj // 8 + 1))
                vector.scalar_tensor_tensor(
                    sv3(ost, 8 * C, (p % 4) * 2 * C, [(C, 2), (1, C)]),
                    sv3(ps, 4096, (i % 4) * 1024, [(1024, 2), (1, 768)]),
                    1.0 / (CVS ** 3),
                    sv3(xtk, NTIL2 * C, i * C, [(C, 2), (1, C)]),
                    op0=AOT.mult, op1=AOT.add).then_inc(s_dve, 1)

    return nc


# ---------------- host side ----------------
_NC_CACHE = {}


def _get_nc():
    if "nc" not in _NC_CACHE:
        _NC_CACHE["nc"] = build()
    return _NC_CACHE["nc"]


def _dr_pack(W):
    """[768(k), M] -> per-partition DR layout [128(ki), pair, s, M] flattened."""
    M = W.shape[1]
    out = np.zeros((128, KC // 2, 2, M), np.float32)
    for pr in range(KC // 2):
        for s in range(2):
            out[:, pr, s, :] = W[pr * 256 + s * 128: pr * 256 + (s + 1) * 128, :]
    return out.reshape(128, KC // 2 * 2 * M)


def _prep_weights(w1, b1, cw, cb, w2, b2, ow1, ob1, ocw, ocb, ow2, ob2):
    w1c = _dr_pack(np.hstack([w1, ow1]) * CVS).astype(F8NP)
    w2c = _dr_pack(np.vstack([w2, ow2]) * CVS).astype(F8NP)
    # diag DR pairs: [128(ki), pr_tot, s, 128(m)] with diagonal per s
    diag = np.zeros((128, NPR_TOT, 2, 128), np.float32)
    wtp = np.zeros((128, (27 + 9) * NG), np.float32)
    eye = np.eye(128, dtype=bool)

    def tapw(w_, tp, main):
        dt, dh, dw = tp
        if main:
            return w_[:, 0, dt + 1, dh + 1, dw + 1]
        return w_[:, 0, 0, dh + 1, dw + 1]

    for br, (pairs, w_, base) in enumerate(
            [(MAIN_PAIRS, cw, 0), (OFF_PAIRS, ocw, NPR_MAIN * NG)]):
        for ip, (tA, tB) in enumerate(pairs):
            for g in range(NG):
                pi = base + ip * NG + g
                vA = tapw(w_, tA, br == 0) * CVS
                diag[:, pi, 0, :][eye] = vA[g * 128:(g + 1) * 128]
                if tB is not None:
                    vB = tapw(w_, tB, br == 0) * CVS
                    diag[:, pi, 1, :][eye] = vB[g * 128:(g + 1) * 128]
    i = 0
    for kd in range(3):
        for kh in range(3):
            for kw in range(3):
                for g in range(NG):
                    wtp[:, i] = cw[g * 128:(g + 1) * 128, 0, kd, kh, kw] * CVS
                    i += 1
    for kh in range(3):
        for kw in range(3):
            for g in range(NG):
                wtp[:, i] = ocw[g * 128:(g + 1) * 128, 0, 0, kh, kw] * CVS
                i += 1
    b1cv = np.ascontiguousarray(
        (np.concatenate([b1, ob1]) * CVS).reshape(KC, 128).T).astype(np.float32)
    cbcv = np.ascontiguousarray(
        (np.concatenate([cb, ocb]) * CVS * CVS).reshape(KC, 128).T).astype(np.float32)
    bias2 = (b2 + ob2).astype(np.float32)
    return dict(w1c=w1c, w2c=w2c,
                diag=diag.reshape(128, NPR_TOT * 2 * 128).astype(F8NP),
                b1c=b1cv, cbc=cbcv, wtp=wtp,
                zeros=np.zeros((1, 3584), F8NP)), bias2


def kernel(**inputs):
    x = np.asarray(inputs["x"], dtype=np.float32)
    Tv = int(np.asarray(inputs["T"]))
    assert Tv == T and x.shape == (128, 197, C)
    wd, bias2 = _prep_weights(
        *[np.asarray(inputs[k], dtype=np.float32) for k in
          ("w1", "b1", "cw", "cb", "w2", "b2", "ow1", "ob1", "ocw", "ocb", "ow2", "ob2")])

    in_maps = []
    for core in range(8):
        xs = x[core * 16:(core + 1) * 16]
        xpat = np.ascontiguousarray(xs[:, 1:, :]).reshape(NPIX, C)
        m = dict(wd)
        m["xT"] = np.ascontiguousarray(xpat.T).astype(F8NP)
        m["xtok"] = (xpat + bias2).astype(np.float32)
        m["xcls"] = np.ascontiguousarray(xs[:, 0, :]).astype(np.float32)
        in_maps.append(m)

    nc = _get_nc()
    res = run_bass_kernel_spmd(nc, in_maps, core_ids=list(range(8)))

    full = np.empty((128, 197, C), np.float32)
    for core in range(8):
        o = res.results[core]["out"]
        full[core * 16:(core + 1) * 16, 0, :] = o[NPIX:NPIX + 16]
        full[core * 16:(core + 1) * 16, 1:, :] = o[:NPIX].reshape(16, 196, C)
    return full



# revision 15
# speedup vs baseline: 1.4060x; 1.4060x over previous
"""TRN2 Bass kernel for nn_Adapter (dense_cnn): ViT adapter with two branches
  main:   h1 = xs@w1+b1 ; y = dwconv3d_3x3x3(h1)+cb ; y@w2+b2
  offset: g = xs@ow1    ; hoff = tdiff(g)+ob1 ; oc = dwconv_1x3x3(hoff)+ocb ; oc@ow2+ob2
  out = x with patch tokens += main + offset   (CLS rows pass through)

Data-parallel over 8 NeuronCores: 2 clips (16 frames) per core; adapter
weights replicated. Per-core kernel (raw bass, explicit semaphores):
  - all three matmul stages fp8-e4m3 DoubleRow; depthwise convs are
    PSUM-accumulated diagonal DR matmuls on PE walking only the 14x14
    plane interiors (rank-5 rhs access patterns)
  - PSUM evicts split ACT/DVE (GPSIMD cannot read PSUM); Pool (gpsimd)
    does the SBUF-only work: halo zero-fills, the temporal-diff pass
    (g[t] := (g[t]+ob1) - g[t-1]), and the +x residual add for half the
    matmul2 tiles (via an ACT-scaled bf16 scratch)
  - weights scaled x16/stage to avoid fp8 subnormals; the 1/16^3 scale
    rides the final evict; x and out kept in bf16 (halves DMA bytes --
    all DMAs serialize on one 360GB/s device)
  - xT is repacked host-side column-slice-major so its 8 DMAs run at
    full descriptor bandwidth and matmul1 streams right behind them;
    diag is repacked (branch, group)-major and split into 6 DMAs so
    each conv group's stationaries land just in time (conv is g-major)

Self-contained: hardcodes shapes for x:[128,197,768], T=8 (asserts).
"""
import numpy as np
import ml_dtypes

import concourse.bass as bass
import concourse.mybir as mybir
from concourse.bass_utils import run_bass_kernel_spmd

F32 = mybir.dt.float32
BF16 = mybir.dt.bfloat16
F8 = mybir.dt.float8e4
AOT = mybir.AluOpType
AFT = mybir.ActivationFunctionType
DR = mybir.MatmulPerfMode.DoubleRow
BF = ml_dtypes.bfloat16
F8NP = ml_dtypes.float8_e4m3

# ---- problem constants (per core) ----
C = 768
CA = 384
T = 8
NPL = 256
CLIPS = 2
NPIX_CLIP = T * 14 * 14
NPIX = CLIPS * NPIX_CLIP
KC = C // 128
NG = CA // 128
H1PAD = NG * CLIPS * (T + 2) * NPL
GPAD = NG * CLIPS * T * NPL
GUARD = NPL
NTIL2 = (NPIX + 127) // 128
M1_CH = 392
OUT_ROWS = NPIX + 16
CVS = 16.0   # weight up-scale per stage; /CVS**3 folded into final evict

# tap (dt, dh, dw) lists grouped by dw so DR pairs share dw (step % 16 == 0)
def _pairs(taps):
    by_dw = {}
    for tp in taps:
        by_dw.setdefault(tp[2], []).append(tp)
    prs = []
    for dw in sorted(by_dw):
        grp = by_dw[dw]
        for i in range(0, len(grp) - 1, 2):
            prs.append((grp[i], grp[i + 1]))
        if len(grp) % 2:
            prs.append((grp[-1], None))
    return prs

MAIN_TAPS = [(kd - 1, kh - 1, kw - 1)
             for kd in range(3) for kh in range(3) for kw in range(3)]
OFF_TAPS = [(0, kh - 1, kw - 1) for kh in range(3) for kw in range(3)]
MAIN_PAIRS = _pairs(MAIN_TAPS)   # 15 (12 pairs + 3 singles)
OFF_PAIRS = _pairs(OFF_TAPS)     # 6 (3 pairs + 3 singles)
NPR_MAIN = len(MAIN_PAIRS)
NPR_OFF = len(OFF_PAIRS)
NPR_TOT = NPR_MAIN * NG + NPR_OFF * NG   # 63

M_ORDER = [3, 4, 5, 0, 1, 2]          # off groups first (feeds the diff)
# m1 chunks j-major so PE streams behind the 8 xT column-slice DMAs
M1_CHUNKS = [(j, mi, m) for j in range(8) for mi, m in enumerate(M_ORDER)]
# conv chunks g-major (each group's diag slice arrives just in time),
# mains first (offs additionally need the Pool diff pass)
CONV_CHUNKS = [(1, g, c, tc)
               for g in range(NG) for c in range(CLIPS) for tc in range(4)]
CONV_CHUNKS += [(0, g, c, tc)
                for g in range(NG) for c in range(CLIPS) for tc in range(4)]


def build():
    nc = bass.Bass()
    xT = nc.declare_dram_parameter("xT", [128, KC * NPIX], F8, isOutput=False)
    xtok = nc.declare_dram_parameter("xtok", [NPIX, C], BF16, isOutput=False)
    xcls = nc.declare_dram_parameter("xcls", [16, C], BF16, isOutput=False)
    w1c = nc.declare_dram_parameter("w1c", [128, KC // 2 * 2 * C], F8, isOutput=False)
    w2c = nc.declare_dram_parameter("w2c", [128, KC // 2 * 2 * C], F8, isOutput=False)
    diag = nc.declare_dram_parameter("diag", [128, NPR_TOT * 2 * 128], F8, isOutput=False)
    b1c = nc.declare_dram_parameter("b1c", [128, KC], F32, isOutput=False)
    cbc = nc.declare_dram_parameter("cbc", [128, KC], F32, isOutput=False)
    zeros = nc.declare_dram_parameter("zeros", [1, 3584], F8, isOutput=False)
    out = nc.declare_dram_parameter("out", [OUT_ROWS, C], BF16, isOutput=True)

    xT_sb = nc.alloc_sbuf_tensor([128, KC * NPIX], F8)   # slice-major [j][pr][s][392]
    w1_sb = nc.alloc_sbuf_tensor([128, KC // 2 * 2 * C], F8)   # [pair][s][m]
    w2_sb = nc.alloc_sbuf_tensor([128, KC // 2 * 2 * C], F8)
    diag_sb = nc.alloc_sbuf_tensor([128, NPR_TOT * 2 * 128], F8)  # [br][g][ip][s][m]
    b1_sb = nc.alloc_sbuf_tensor([128, KC], F32)
    cb_sb = nc.alloc_sbuf_tensor([128, KC], F32)
    h1p = nc.alloc_sbuf_tensor([128, H1PAD + 2 * GUARD], F8)
    gp = nc.alloc_sbuf_tensor([128, GPAD + 2 * GUARD], F8)
    cvo = nc.alloc_sbuf_tensor([128, KC * NPIX], F8)
    xtk = nc.alloc_sbuf_tensor([128, NTIL2 * C], BF16)
    ost = nc.alloc_sbuf_tensor([128, 8 * C], BF16)
    sc2 = nc.alloc_sbuf_tensor([128, 2 * C], BF16)      # ACT->Pool m2 scratch
    ps = nc.alloc_psum_tensor([128, 4096], F32)

    XT_EXT = KC * NPIX
    W_EXT = KC // 2 * 2 * C
    DG_EXT = NPR_TOT * 2 * 128
    H1_EXT = H1PAD + 2 * GUARD
    GP_EXT = GPAD + 2 * GUARD
    CV_EXT = KC * NPIX
    PS_EXT = 4096

    def h1_plane(g, c, tpad):
        return GUARD + ((g * CLIPS + c) * (T + 2) + tpad) * NPL

    def g_plane(g, c, t):
        return GUARD + ((g * CLIPS + c) * T + t) * NPL

    def diag_off(br, g, ip):
        """branch/group-major diag tile offset (br 1=main first)."""
        if br:
            return (g * NPR_MAIN + ip) * 256
        return (NG * NPR_MAIN + g * NPR_OFF + ip) * 256

    # ---------- static per-engine op schedules ----------
    # keys: ("halo",i) ("m1ev",n) ("diff",g,c) ("cvev",k) ("m2ev",i)
    #       ("m2act",i) ("m2add",i)
    # m2 tile paths: D = DVE full STT; AP = ACT scale + Pool add;
    # AD = ACT scale + DVE bf16 add (Pool cannot STT; balance the three)
    M2_PATH = ["D", "AP", "AD", "D", "AP",
               "D", "AD", "AP", "D", "AD",
               "D", "AP", "AD", "D", "AP",
               "D", "AD", "AP", "D", "AD",
               "D", "AP", "AD", "D", "AP"]
    SCR_USERS = [i for i in range(NTIL2) if M2_PATH[i] != "D"]
    SCR_SLOT = {i: u % 2 for u, i in enumerate(SCR_USERS)}
    # final (ost-writing) op per m2 tile, for out-DMA and psum-reuse waits
    FIN = {i: (("m2ev", i) if M2_PATH[i] == "D" else ("m2add", i))
           for i in range(NTIL2)}

    act_ops = [("m1ev", n) for n in range(48) if n % 2 == 0]
    act_ops += [("cvev", k) for k in range(48) if k % 2 == 0]
    act_ops += [("m2act", i) for i in range(NTIL2) if M2_PATH[i] != "D"]
    dve_ops = [("m1ev", n) for n in range(48) if n % 2 == 1]
    # interleave the 6 diff blocks into the conv-evict stream (off-branch
    # conv only needs them from chunk 24 onward; this avoids PE bank stalls)
    cvd = [("cvev", k) for k in range(48) if k % 2 == 1]
    for d, (g, c) in enumerate(((0, 0), (1, 0), (2, 0), (0, 1), (1, 1), (2, 1))):
        cvd.insert(2 * d + 1, ("diff", g, c))
    dve_ops += cvd
    dve_ops += [(("m2ev", i) if M2_PATH[i] == "D" else ("m2add", i))
                for i in range(NTIL2) if M2_PATH[i] in ("D", "AD")]
    pool_ops = [("halo", i) for i in range(8)]       # 0..3 h1p, 4..7 gp
    pool_ops += [("m2add", i) for i in range(NTIL2) if M2_PATH[i] == "AP"]

    ev_idx = {}   # op key -> (engine, 1-based index)
    for eng, ops in (("A", act_ops), ("P", pool_ops), ("D", dve_ops)):
        for pos, key in enumerate(ops):
            ev_idx[key] = (eng, pos + 1)

    def chunk_jc(n):
        return M1_CHUNKS[n]

    def m1_main_chunk(g, c, tc):
        j = c * 4 + min(tc + 1, 3)
        return j * 6 + (3 + g)       # main group g sits at M_ORDER index 3+g

    # conv-evict coverage needed by m2 tile i: all chunks with (c,tc) below
    def cov_need(i):
        p_hi = min(128 * (i + 1), NPIX) - 1
        c_hi = p_hi // NPIX_CLIP
        t_hi = (p_hi % NPIX_CLIP) // 196
        return c_hi, t_hi // 2

    def pe_thr(key):
        if key[0] == "m1ev":
            return key[1] + 1
        if key[0] == "cvev":
            return 48 + key[1] + 1
        return 96 + key[1] + 1

    with (
        nc.Block() as block,
        nc.semaphore("s_pe") as s_pe,
        nc.semaphore("s_evA") as s_evA,
        nc.semaphore("s_evP") as s_evP,
        nc.semaphore("s_evD") as s_evD,
        nc.semaphore("s_xt") as s_xt,
        nc.semaphore("s_w1") as s_w1,
        nc.semaphore("s_w2") as s_w2,
        nc.semaphore("s_dg") as s_dg,
        nc.semaphore("s_ld") as s_ld,
        nc.semaphore("s_z") as s_z,
        nc.semaphore("s_xk") as s_xk,
        nc.semaphore("s_ot") as s_ot,
    ):
        SEM = {"A": s_evA, "P": s_evP, "D": s_evD}

        def wait_ev(h, key):
            e, t = ev_idx[key]
            h.wait_ge(SEM[e], t)

        # ================= SP: all DMA =================
        @block.sync
        def _(sync):
            sync.dma_start(out=w1_sb[:], in_=w1c[:]).then_inc(s_w1, 16)
            for j in (0, 1, 2, 3):
                sync.dma_start(out=xT_sb[:, j * 2352:(j + 1) * 2352],
                               in_=xT[:, j * 2352:(j + 1) * 2352]).then_inc(s_xt, 16)
            sync.dma_start(out=b1_sb[:], in_=b1c[:]).then_inc(s_ld, 16)
            sync.dma_start(out=cb_sb[:], in_=cbc[:]).then_inc(s_ld, 16)
            for j in (4, 5, 6, 7):
                sync.dma_start(out=xT_sb[:, j * 2352:(j + 1) * 2352],
                               in_=xT[:, j * 2352:(j + 1) * 2352]).then_inc(s_xt, 16)
            # zero-fill: gp guards (2 runs) + h1p t-halo/guard runs (7x512)
            sync.dma_start(
                out=bass.AP(gp, 0, [[GP_EXT, 128], [GUARD + GPAD, 2], [1, GUARD]]),
                in_=bass.AP(zeros, 0, [[0, 128], [GUARD, 2], [1, GUARD]]),
            ).then_inc(s_z, 16)
            sync.dma_start(
                out=bass.AP(h1p, 0, [[H1_EXT, 128], [2560, 7], [1, 512]]),
                in_=bass.AP(zeros, 0, [[0, 128], [512, 7], [1, 512]]),
            ).then_inc(s_z, 16)
            for g in range(NG):      # main diag, one group at a time
                o = diag_off(1, g, 0)
                sync.dma_start(out=diag_sb[:, o:o + NPR_MAIN * 256],
                               in_=diag[:, o:o + NPR_MAIN * 256]).then_inc(s_dg, 16)
            for g in range(NG):      # offset diag
                o = diag_off(0, g, 0)
                sync.dma_start(out=diag_sb[:, o:o + NPR_OFF * 256],
                               in_=diag[:, o:o + NPR_OFF * 256]).then_inc(s_dg, 16)
            sync.dma_start(out=w2_sb[:], in_=w2c[:]).then_inc(s_w2, 16)
            sync.dma_start(out=out[NPIX:OUT_ROWS, :], in_=xcls[:]).then_inc(s_z, 16)
            for pj in range(12):     # all xtok pair-loads up front
                j = 2 * pj
                sync.dma_start(
                    out=xtk[:, j * C:(j + 2) * C].rearrange("p (b c) -> p b c", b=2),
                    in_=xtok[j * 128:(j + 2) * 128, :].rearrange("(b r) c -> r b c", b=2),
                ).then_inc(s_xk, 16)
            sync.dma_start(out=xtk[:64, bass.ts(24, C)],
                           in_=xtok[24 * 128:NPIX, :]).then_inc(s_xk, 16)
            for p in range(NTIL2 // 2):          # 12 pairs
                i = 2 * p
                wait_ev(sync, FIN[i])
                wait_ev(sync, FIN[i + 1])
                sync.dma_start(
                    out=out[i * 128:(i + 2) * 128, :].rearrange("(b r) c -> r b c", b=2),
                    in_=ost[:, (i % 8) * C:((i % 8) + 2) * C].rearrange("p (b c) -> p b c", b=2),
                ).then_inc(s_ot, 16)
            wait_ev(sync, FIN[24])
            sync.dma_start(out=out[24 * 128:NPIX, :],
                           in_=ost[:64, bass.ts(24 % 8, C)]).then_inc(s_ot, 16)

        # ================= PE =================
        @block.tensor
        def _(tensor):
            tensor.wait_ge(s_w1, 16)
            # ---- matmul1 (DR, 3 pair-passes), banks 0..7 rotating ----
            for n, (j, mi, m) in enumerate(M1_CHUNKS):
                bank = n % 8
                if mi == 0:
                    tensor.wait_ge(s_xt, 16 * (j + 1))
                if n >= 8:
                    wait_ev(tensor, ("m1ev", n - 8))
                pv = ps[:, bank * 512: bank * 512 + M1_CH]
                for pr in range(KC // 2):
                    lhsT = bass.AP(w1_sb, pr * 2 * C + m * 128,
                                   [[W_EXT, 128], [C, 2], [1, 128]])
                    rhs = bass.AP(xT_sb, j * 2352 + pr * 2 * M1_CH,
                                  [[XT_EXT, 128], [M1_CH, 2], [1, M1_CH]])
                    mm = tensor.matmul(pv, lhsT, rhs, perf_mode=DR,
                                       start=(pr == 0), stop=(pr == KC // 2 - 1))
                mm.then_inc(s_pe, 1)
            # ---- conv (DR pairs, interior walk), banks 4..7 rotating ----
            tensor.wait_ge(s_z, 32)
            for k, (br, g, c, tc) in enumerate(CONV_CHUNKS):
                bank = 4 + k % 4
                if k == 0:
                    wait_ev(tensor, ("halo", 3))       # h1p halos done
                if k % 8 == 0:                          # new (br, g) section
                    sec = g + 1 if br else 3 + g + 1
                    tensor.wait_ge(s_dg, 16 * sec)
                if k >= 4:
                    wait_ev(tensor, ("cvev", k - 4))
                else:
                    wait_ev(tensor, ("m1ev", 44 + k))
                if br:
                    wait_ev(tensor, ("m1ev", m1_main_chunk(g, c, tc)))
                else:
                    if k == 24:
                        wait_ev(tensor, ("halo", 7))    # gp halos done
                    wait_ev(tensor, ("diff", g, c))
                pairs = MAIN_PAIRS if br else OFF_PAIRS
                for ip, (tA, tB) in enumerate(pairs):
                    dtA, dhA, dwA = tA
                    if br:
                        offA = h1_plane(g, c, 2 * tc + 1 + dtA) + (1 + dhA) * 16 + (1 + dwA)
                        buf, ext = h1p, H1_EXT
                    else:
                        offA = g_plane(g, c, 2 * tc) + (1 + dhA) * 16 + (1 + dwA)
                        buf, ext = gp, GP_EXT
                    if tB is None:
                        sstep = 16
                    else:
                        dtB, dhB, dwB = tB
                        sstep = (dtB - dtA) * 256 + (dhB - dhA) * 16
                    lhsT = bass.AP(diag_sb, diag_off(br, g, ip),
                                   [[DG_EXT, 128], [128, 2], [1, 128]])
                    for pl in range(2):      # one matmul per t-plane (3D free cap)
                        rhs = bass.AP(buf, offA + pl * 256,
                                      [[ext, 128], [sstep, 2], [16, 14], [1, 14]])
                        pv = ps[:, bank * 512 + pl * 196: bank * 512 + pl * 196 + 196]
                        mm = tensor.matmul(pv, lhsT, rhs, perf_mode=DR,
                                           start=(ip == 0),
                                           stop=(ip == len(pairs) - 1),
                                           skip_group_check=True)
                mm.then_inc(s_pe, 1)
            # ---- matmul2 (DR, token-major), psum slots {0,1}/{2,3} ----
            tensor.wait_ge(s_w2, 16)
            for i in range(NTIL2):
                rows = min(128, NPIX - i * 128)
                slot = i % 4
                c_hi, tc_hi = cov_need(i)
                for eng, ops in (("A", act_ops), ("P", pool_ops), ("D", dve_ops)):
                    last = None
                    for pos, key in enumerate(ops):
                        if key[0] == "cvev":
                            _, gk, ck, tck = CONV_CHUNKS[key[1]]
                            if (ck, tck) <= (c_hi, tc_hi):
                                last = pos + 1
                    if last is not None:
                        tensor.wait_ge(SEM[eng], last)
                if i >= 4:
                    wait_ev(tensor, FIN[i - 4])
                elif i < 2:
                    wait_ev(tensor, ("m1ev", 40 + 2 * i))
                    wait_ev(tensor, ("m1ev", 41 + 2 * i))
                else:
                    wait_ev(tensor, ("cvev", 44 + 2 * (i - 2)))
                    wait_ev(tensor, ("cvev", 45 + 2 * (i - 2)))
                pv = ps[:rows, slot * 1024: slot * 1024 + 768]
                for pr in range(KC // 2):
                    lhsT = bass.AP(cvo, (pr * 2) * NPIX + i * 128,
                                   [[CV_EXT, 128], [NPIX, 2], [1, rows]])
                    tensor.matmul(pv[:, 0:512], lhsT,
                                  bass.AP(w2_sb, pr * 2 * C,
                                          [[W_EXT, 128], [C, 2], [1, 512]]),
                                  perf_mode=DR,
                                  start=(pr == 0), stop=(pr == KC // 2 - 1),
                                  skip_group_check=True)
                    mm1 = tensor.matmul(pv[:, 512:768], lhsT,
                                        bass.AP(w2_sb, pr * 2 * C + 512,
                                                [[W_EXT, 128], [C, 2], [1, 256]]),
                                        perf_mode=DR,
                                        start=(pr == 0), stop=(pr == KC // 2 - 1),
                                        skip_group_check=True)
                mm1.then_inc(s_pe, 1)

        # ---------- shared evict-op helpers ----------
        def m1_src(n):
            return bass.AP(ps, (n % 8) * 512,
                           [[PS_EXT, 128], [196, 2], [14, 14], [1, 14]])

        def m1_dst(n):
            j, mi, m = chunk_jc(n)
            c, t = divmod(2 * j, T)
            if m < 3:
                return bass.AP(h1p, h1_plane(m, c, t + 1) + 17,
                               [[H1_EXT, 128], [256, 2], [16, 14], [1, 14]])
            return bass.AP(gp, g_plane(m - 3, c, t) + 17,
                           [[GP_EXT, 128], [256, 2], [16, 14], [1, 14]])

        def cv_src(k):
            return bass.AP(ps, (4 + k % 4) * 512, [[PS_EXT, 128], [1, 392]])

        def cv_dst(k):
            br, g, c, tc = CONV_CHUNKS[k]
            grp = g if br else 3 + g
            off = grp * NPIX + c * NPIX_CLIP + 2 * tc * 196
            return cvo[:, off: off + 392]

        # ================= ACT =================
        @block.scalar
        def _(scalar):
            scalar.wait_ge(s_ld, 32)
            for key in act_ops:
                scalar.wait_ge(s_pe, pe_thr(key))
                if key[0] == "m1ev":
                    n = key[1]
                    m = chunk_jc(n)[2]
                    bias = b1_sb[:, m:m + 1] if m < 3 else 0.0
                    scalar.activation(m1_dst(n), m1_src(n), AFT.Identity,
                                      bias=bias).then_inc(s_evA, 1)
                elif key[0] == "cvev":
                    k = key[1]
                    br, g, c, tc = CONV_CHUNKS[k]
                    grp = g if br else 3 + g
                    scalar.activation(cv_dst(k), cv_src(k), AFT.Identity,
                                      bias=cb_sb[:, grp:grp + 1]).then_inc(s_evA, 1)
                else:                       # m2act: psum -> bf16 scratch, x1/4096
                    i = key[1]
                    rows = min(128, NPIX - i * 128)
                    u = SCR_USERS.index(i)
                    if u >= 2:
                        wait_ev(scalar, ("m2add", SCR_USERS[u - 2]))  # slot free
                    scalar.activation(
                        sc2[:rows, SCR_SLOT[i] * C:SCR_SLOT[i] * C + C],
                        ps[:rows, (i % 4) * 1024:(i % 4) * 1024 + 768],
                        AFT.Identity, scale=1.0 / (CVS ** 3)).then_inc(s_evA, 1)

        # ================= Pool (gpsimd): SBUF-only work =================
        @block.gpsimd
        def _(pool):
            for key in pool_ops:
                if key[0] == "halo":
                    hi = key[1]
                    buf, ext, npl_ = ((h1p, H1_EXT, 60), (gp, GP_EXT, 48))[hi // 4]
                    off, dims = (
                        (GUARD, [[256, npl_], [1, 16]]),           # row 0
                        (GUARD + 240, [[256, npl_], [1, 16]]),     # row 15
                        (GUARD, [[256, npl_], [16, 16]]),          # col 0
                        (GUARD + 15, [[256, npl_], [16, 16]]),     # col 15
                    )[hi % 4]
                    pool.memset(bass.AP(buf, off, [[ext, 128]] + dims),
                                0.0).then_inc(s_evP, 1)
                else:                       # m2add: ost = scratch + xtk
                    i = key[1]
                    rows = min(128, NPIX - i * 128)
                    wait_ev(pool, ("m2act", i))
                    pool.wait_ge(s_xk, 16 * (i // 2 + 1))
                    if i >= 8:
                        pool.wait_ge(s_ot, 16 * ((i - 8) // 2 + 1))
                    pool.tensor_tensor(
                        ost[:rows, (i % 8) * C:(i % 8) * C + C],
                        sc2[:rows, SCR_SLOT[i] * C:SCR_SLOT[i] * C + C],
                        xtk[:rows, i * C:i * C + C],
                        op=AOT.add).then_inc(s_evP, 1)

        # ================= DVE =================
        @block.vector
        def _(vector):
            vector.wait_ge(s_ld, 32)
            for key in dve_ops:
                if key[0] == "m1ev":
                    n = key[1]
                    vector.wait_ge(s_pe, pe_thr(key))
                    m = chunk_jc(n)[2]
                    bias = b1_sb[:, m:m + 1] if m < 3 else 0.0
                    vector.tensor_scalar(m1_dst(n), m1_src(n), bias, None,
                                         op0=AOT.add).then_inc(s_evD, 1)
                elif key[0] == "cvev":
                    k = key[1]
                    vector.wait_ge(s_pe, pe_thr(key))
                    br, g, c, tc = CONV_CHUNKS[k]
                    grp = g if br else 3 + g
                    vector.tensor_scalar(cv_dst(k), cv_src(k),
                                         cb_sb[:, grp:grp + 1], None,
                                         op0=AOT.add).then_inc(s_evD, 1)
                elif key[0] == "diff":
                    _, g, c = key
                    Q = (c * 4 + 3) * 6 + g
                    cnt = sum(1 for kk in act_ops
                              if kk[0] == "m1ev" and kk[1] <= Q)
                    vector.wait_ge(s_evA, cnt)
                    ob1 = b1_sb[:, 3 + g:4 + g]
                    iv = [[GP_EXT, 128], [16, 14], [1, 14]]
                    for t in range(T - 1, 0, -1):
                        a = g_plane(g, c, t) + 17
                        b = g_plane(g, c, t - 1) + 17
                        vector.scalar_tensor_tensor(
                            bass.AP(gp, a, iv), bass.AP(gp, a, iv), ob1,
                            bass.AP(gp, b, iv),
                            op0=AOT.add, op1=AOT.subtract)
                    z = g_plane(g, c, 0) + 17
                    vector.tensor_scalar(
                        bass.AP(gp, z, iv), bass.AP(gp, z, iv), 0.0, ob1,
                        op0=AOT.mult, op1=AOT.add).then_inc(s_evD, 1)
                elif key[0] == "m2ev":      # full STT evict + residual
                    i = key[1]
                    rows = min(128, NPIX - i * 128)
                    vector.wait_ge(s_pe, pe_thr(key))
                    vector.wait_ge(s_xk, 16 * (i // 2 + 1))
                    if i >= 8:
                        vector.wait_ge(s_ot, 16 * ((i - 8) // 2 + 1))
                    vector.scalar_tensor_tensor(
                        ost[:rows, (i % 8) * C:(i % 8) * C + C],
                        ps[:rows, (i % 4) * 1024:(i % 4) * 1024 + 768],
                        1.0 / (CVS ** 3),
                        xtk[:rows, i * C:i * C + C],
                        op0=AOT.mult, op1=AOT.add).then_inc(s_evD, 1)
                else:                       # m2add (AD path): bf16 TT add
                    i = key[1]
                    rows = min(128, NPIX - i * 128)
                    wait_ev(vector, ("m2act", i))
                    vector.wait_ge(s_xk, 16 * (i // 2 + 1))
                    if i >= 8:
                        vector.wait_ge(s_ot, 16 * ((i - 8) // 2 + 1))
                    vector.tensor_tensor(
                        ost[:rows, (i % 8) * C:(i % 8) * C + C],
                        sc2[:rows, SCR_SLOT[i] * C:SCR_SLOT[i] * C + C],
                        xtk[:rows, i * C:i * C + C],
                        op=AOT.add).then_inc(s_evD, 1)

    return nc


# ---------------- host side ----------------
_NC_CACHE = {}


def _get_nc():
    if "nc" not in _NC_CACHE:
        _NC_CACHE["nc"] = build()
    return _NC_CACHE["nc"]


def _dr_pack(W):
    """[768(k), M] -> per-partition DR layout [128(ki), pair, s, M] flattened."""
    M = W.shape[1]
    out = np.zeros((128, KC // 2, 2, M), np.float32)
    for pr in range(KC // 2):
        for s in range(2):
            out[:, pr, s, :] = W[pr * 256 + s * 128: pr * 256 + (s + 1) * 128, :]
    return out.reshape(128, KC // 2 * 2 * M)


def _prep_weights(w1, b1, cw, cb, w2, b2, ow1, ob1, ocw, ocb, ow2, ob2):
    w1c = _dr_pack(np.hstack([w1, ow1]) * CVS).astype(F8NP)
    w2c = _dr_pack(np.vstack([w2, ow2]) * CVS).astype(F8NP)
    # diag tiles, (branch, group)-major: [128(ki), br/g/ip, s, 128(m)]
    diag = np.zeros((128, NPR_TOT, 2, 128), np.float32)
    eye = np.eye(128, dtype=bool)

    def tapw(w_, tp, main):
        dt, dh, dw = tp
        if main:
            return w_[:, 0, dt + 1, dh + 1, dw + 1]
        return w_[:, 0, 0, dh + 1, dw + 1]

    for main, (pairs, w_) in ((True, (MAIN_PAIRS, cw)), (False, (OFF_PAIRS, ocw))):
        for ip, (tA, tB) in enumerate(pairs):
            for g in range(NG):
                if main:
                    pi = g * NPR_MAIN + ip
                else:
                    pi = NG * NPR_MAIN + g * NPR_OFF + ip
                vA = tapw(w_, tA, main) * CVS
                diag[:, pi, 0, :][eye] = vA[g * 128:(g + 1) * 128]
                if tB is not None:
                    vB = tapw(w_, tB, main) * CVS
                    diag[:, pi, 1, :][eye] = vB[g * 128:(g + 1) * 128]
    b1cv = np.ascontiguousarray(
        (np.concatenate([b1, ob1]) * CVS).reshape(KC, 128).T).astype(np.float32)
    cbcv = np.ascontiguousarray(
        (np.concatenate([cb, ocb]) * CVS * CVS).reshape(KC, 128).T).astype(np.float32)
    bias2 = (b2 + ob2).astype(np.float32)
    return dict(w1c=w1c, w2c=w2c,
                diag=diag.reshape(128, NPR_TOT * 2 * 128).astype(F8NP),
                b1c=b1cv, cbc=cbcv,
                zeros=np.zeros((1, 3584), F8NP)), bias2


def _pack_xT(xpat):
    """[NPIX, C] f32 -> slice-major [128, j(8) k(6) 392] fp8."""
    A = np.ascontiguousarray(xpat.T).astype(F8NP)           # [768, 3136]
    A = A.reshape(KC, 128, 8, M1_CH)                        # [k, p, j, col]
    return np.ascontiguousarray(
        A.transpose(1, 2, 0, 3).reshape(128, KC * NPIX))    # [p, j, k, col]


def kernel(**inputs):
    x = np.asarray(inputs["x"], dtype=np.float32)
    Tv = int(np.asarray(inputs["T"]))
    assert Tv == T and x.shape == (128, 197, C)
    wd, bias2 = _prep_weights(
        *[np.asarray(inputs[k], dtype=np.float32) for k in
          ("w1", "b1", "cw", "cb", "w2", "b2", "ow1", "ob1", "ocw", "ocb", "ow2", "ob2")])

    in_maps = []
    for core in range(8):
        xs = x[core * 16:(core + 1) * 16]
        xpat = np.ascontiguousarray(xs[:, 1:, :]).reshape(NPIX, C)
        m = dict(wd)
        m["xT"] = _pack_xT(xpat)
        m["xtok"] = (xpat + bias2).astype(BF)
        m["xcls"] = np.ascontiguousarray(xs[:, 0, :]).astype(BF)
        in_maps.append(m)

    nc = _get_nc()
    res = run_bass_kernel_spmd(nc, in_maps, core_ids=list(range(8)))

    full = np.empty((128, 197, C), np.float32)
    for core in range(8):
        o = np.asarray(res.results[core]["out"], dtype=np.float32)
        full[core * 16:(core + 1) * 16, 0, :] = o[NPIX:NPIX + 16]
        full[core * 16:(core + 1) * 16, 1:, :] = o[:NPIX].reshape(16, 196, C)
    return full


# revision 47
# speedup vs baseline: 1.5587x; 1.1086x over previous
"""TRN2 Bass kernel for nn_Adapter (dense_cnn): ViT adapter with two branches
  main:   h1 = xs@w1+b1 ; y = dwconv3d_3x3x3(h1)+cb ; y@w2+b2
  offset: g = xs@ow1    ; hoff = tdiff(g)+ob1 ; oc = dwconv_1x3x3(hoff)+ocb ; oc@ow2+ob2
  out = x with patch tokens += main + offset   (CLS rows pass through)

Data-parallel over 8 NeuronCores: 2 clips (16 frames) per core; adapter
weights replicated. Per-core kernel (raw bass, explicit semaphores):
  - all three matmul stages fp8-e4m3 DoubleRow; depthwise convs are
    PSUM-accumulated diagonal DR matmuls on PE walking only the 14x14
    plane interiors (rank-4 access patterns, one matmul per t-plane)
  - matmul2 token-tiles are interleaved into the conv chunk stream as
    their cvo coverage completes, so PE never idles between phases;
    matmul2 uses three 768-wide psum slots (3x256 segments, bank-safe)
  - PSUM evicts split ACT/DVE (GPSIMD cannot read PSUM); every matmul2
    evict is a single DVE scalar_tensor_tensor fusing the 1/16^3 scale
    with the +x residual; Pool (gpsimd) zero-fills the conv halos
  - temporal diff writes a separate gd buffer (one fused 7-plane
    scalar_tensor_tensor per (group, clip): gd[t] = (g[t]+ob1) - g[t-1])
  - x and out in bf16 (halves DMA bytes; DMAs serialize at 360GB/s);
    xT repacked host-side column-slice-major so matmul1 streams right
    behind its 8 slice DMAs at full descriptor bandwidth

Self-contained: hardcodes shapes for x:[128,197,768], T=8 (asserts).
"""
import numpy as np
import ml_dtypes

import concourse.bass as bass
import concourse.mybir as mybir
from concourse.bass_utils import run_bass_kernel_spmd

F32 = mybir.dt.float32
BF16 = mybir.dt.bfloat16
F8 = mybir.dt.float8e4
AOT = mybir.AluOpType
AFT = mybir.ActivationFunctionType
DR = mybir.MatmulPerfMode.DoubleRow
BF = ml_dtypes.bfloat16
F8NP = ml_dtypes.float8_e4m3

# ---- problem constants (per core) ----
C = 768
CA = 384
T = 8
NPL = 256
CLIPS = 2
NPIX_CLIP = T * 14 * 14
NPIX = CLIPS * NPIX_CLIP
KC = C // 128
NG = CA // 128
H1PAD = NG * CLIPS * (T + 2) * NPL
GPAD = NG * CLIPS * T * NPL
GUARD = NPL
NTIL2 = (NPIX + 127) // 128
M1_CH = 392
OUT_ROWS = NPIX + 16
CVS = 16.0   # weight up-scale per stage; /CVS**3 folded into final evict

# tap (dt, dh, dw) lists grouped by dw so DR pairs share dw (step % 16 == 0)
def _pairs(taps):
    by_dw = {}
    for tp in taps:
        by_dw.setdefault(tp[2], []).append(tp)
    prs = []
    for dw in sorted(by_dw):
        grp = by_dw[dw]
        for i in range(0, len(grp) - 1, 2):
            prs.append((grp[i], grp[i + 1]))
        if len(grp) % 2:
            prs.append((grp[-1], None))
    return prs

MAIN_TAPS = [(kd - 1, kh - 1, kw - 1)
             for kd in range(3) for kh in range(3) for kw in range(3)]
OFF_TAPS = [(0, kh - 1, kw - 1) for kh in range(3) for kw in range(3)]
MAIN_PAIRS = _pairs(MAIN_TAPS)   # 15 (12 pairs + 3 singles)
OFF_PAIRS = _pairs(OFF_TAPS)     # 6 (3 pairs + 3 singles)
NPR_MAIN = len(MAIN_PAIRS)
NPR_OFF = len(OFF_PAIRS)
NPR_TOT = NPR_MAIN * NG + NPR_OFF * NG   # 63

M_ORDER = [3, 4, 5, 0, 1, 2]          # off groups first (feeds the diff)
# m1 chunks j-major so PE streams behind the 8 xT column-slice DMAs
M1_CHUNKS = [(j, mi, m) for j in range(8) for mi, m in enumerate(M_ORDER)]
# conv chunks (c,tc)-major so m2 tiles can interleave as coverage lands
CONV_CHUNKS = [(br, g, c, tc)
               for c in range(CLIPS) for tc in range(4)
               for br in (1, 0) for g in range(NG)]
# m2 tiles ready after conv block B (6 chunks each): 392*(B+1) pixels
M2_READY = [min(NTIL2, (392 * (B + 1)) // 128) for B in range(8)]
M2_READY[7] = NTIL2
# m2 psum slots: banks 0..3 fit two non-overlapping 768-wide slots; the
# tail tiles (after conv coverage) use two more in the freed conv banks
M2_SLOTS = [0, 768]
# m1 bank map: banks 4..7 see their last use by chunk 35 so the conv chunks
# (banks 4..7) start right at m1 end; chunks 36+ cycle banks 0..3
M1_BANK = [(n + 4) % 8 if n < 36 else [0, 1, 2, 3, 6, 7][(n - 36) % 6]
           for n in range(48)]
M1_PREV = {}
for n, b in enumerate(M1_BANK):
    for n2 in range(n + 1, 48):
        if M1_BANK[n2] == b:
            M1_PREV[n2] = n
            break
M1_LAST = {b: max(n for n in range(48) if M1_BANK[n] == b) for b in range(8)}


def build(warmup_n=0):
    global WARMUP_N
    WARMUP_N = warmup_n
    nc = bass.Bass()
    xT = nc.declare_dram_parameter("xT", [128, KC * NPIX], F8, isOutput=False)
    xtok = nc.declare_dram_parameter("xtok", [NPIX, C], BF16, isOutput=False)
    xcls = nc.declare_dram_parameter("xcls", [16, C], BF16, isOutput=False)
    w1c = nc.declare_dram_parameter("w1c", [128, KC // 2 * 2 * C], F8, isOutput=False)
    w2c = nc.declare_dram_parameter("w2c", [128, KC // 2 * 2 * C], F8, isOutput=False)
    diag = nc.declare_dram_parameter("diag", [128, NPR_TOT * 2 * 128], F8, isOutput=False)
    b1c = nc.declare_dram_parameter("b1c", [128, KC], F32, isOutput=False)
    cbc = nc.declare_dram_parameter("cbc", [128, KC], F32, isOutput=False)
    zeros = nc.declare_dram_parameter("zeros", [1, 3584], F8, isOutput=False)
    eyec = nc.declare_dram_parameter("eyec", [128, 128], BF16, isOutput=False)
    out = nc.declare_dram_parameter("out", [OUT_ROWS, C], BF16, isOutput=True)

    xT_sb = nc.alloc_sbuf_tensor([128, KC * NPIX], F8)   # slice-major [j][pr][s][392]
    w1_sb = nc.alloc_sbuf_tensor([128, KC // 2 * 2 * C], F8)   # [pair][s][m]
    w2_sb = nc.alloc_sbuf_tensor([128, KC // 2 * 2 * C], F8)
    diag_sb = nc.alloc_sbuf_tensor([128, NPR_TOT * 2 * 128], F8)  # [br][g][ip][s][m]
    b1_sb = nc.alloc_sbuf_tensor([128, KC], F32)
    cb_sb = nc.alloc_sbuf_tensor([128, KC], F32)
    h1p = nc.alloc_sbuf_tensor([128, H1PAD + 2 * GUARD], F8)
    gp = nc.alloc_sbuf_tensor([128, GPAD], F8)           # raw off-branch m1 out
    gd = nc.alloc_sbuf_tensor([128, GPAD + GUARD], F8)   # diffed planes + tail
    cvo = nc.alloc_sbuf_tensor([128, KC * NPIX], F8)
    xtk = nc.alloc_sbuf_tensor([128, NTIL2 * C], BF16)
    ost = nc.alloc_sbuf_tensor([128, 8 * C], BF16)
    zsb = nc.alloc_sbuf_tensor([128, 960], F8)
    eye_sb = nc.alloc_sbuf_tensor([128, 128], BF16)     # 4096*I for +x accumulate
    ps = nc.alloc_psum_tensor([128, 4096], F32)

    XT_EXT = KC * NPIX
    W_EXT = KC // 2 * 2 * C
    DG_EXT = NPR_TOT * 2 * 128
    H1_EXT = H1PAD + 2 * GUARD
    GP_EXT = GPAD
    GD_EXT = GPAD + GUARD
    CV_EXT = KC * NPIX
    PS_EXT = 4096

    def h1_plane(g, c, tpad):
        return GUARD + ((g * CLIPS + c) * (T + 2) + tpad) * NPL

    def g_plane(g, c, t):
        return ((g * CLIPS + c) * T + t) * NPL

    def diag_off(br, g, ip):
        """branch/group-major diag tile offset (br 1=main first)."""
        if br:
            return (g * NPR_MAIN + ip) * 256
        return (NG * NPR_MAIN + g * NPR_OFF + ip) * 256

    # ---------- static schedules ----------
    BISECT_NO_ILV = globals().get("_BISECT_NO_ILV", False)
    # PE psum-op order: m1 chunks, then conv chunks with m2 tiles interleaved
    # one block behind their coverage (so the block's evicts are already done)
    pe_seq = [("m1", n) for n in range(48)]
    ti = 0
    for B in range(8):
        for idx, k in enumerate(range(6 * B, 6 * (B + 1))):
            pe_seq.append(("cv", k))
            if (not BISECT_NO_ILV) and idx % 2 == 1 and B > 0 \
                    and ti < M2_READY[B - 1]:
                pe_seq.append(("m2", ti))
                ti += 1
    while ti < 21:
        pe_seq.append(("m2", ti))
        ti += 1
    pe_seq += [("m2", 21), ("m2", 22), ("m2", 23), ("m2", 24)]
    pe_ord = {key: pos + 1 for pos, key in enumerate(pe_seq)}
    ID_TILES = (21, 23)           # +4096x via PE identity matmul, ACT evict
    FIN = {i: (("m2act", i) if i in ID_TILES else ("m2ev", i))
           for i in range(NTIL2)}
    # tail tiles use the conv banks (4..7, free once coverage is waited);
    # tile 24 reuses slot 0, whose previous user is tile 18
    SLOT_OF = {i: M2_SLOTS[i % 2] for i in range(21)}
    SLOT_OF.update({21: 2048, 22: 2816, 23: 0, 24: 768})

    # evict-op keys: ("halo",i) ("m1ev",n) ("diffp",g,c,t) ("cvev",k) ("m2ev",i)
    ACT_M1 = [n for n in range(48) if n % 2 == 0]
    ACT_CV = sorted([k for k in range(48) if k % 2 == 0] + [1, 3])

    act_ops = [("m1ev", n) for n in ACT_M1]
    act_ops += [("cvev", k) for k in ACT_CV]
    act_ops += [("m2act", i) for i in ID_TILES]
    # DVE: m1 evicts (odd thru 39), then the c=0 diffs plane-by-plane
    # (t ascending; off-conv (g,c,tc) waits plane 2tc+1), then the conv/m2
    # evict stream with c=1 diffs slotted in after early m2 tiles
    dve_ops = [("m1ev", n) for n in range(48) if n % 2 == 1]
    dve_ops += [("diffp", g, 0, t) for g in range(NG) for t in range(T)]
    stream = [("cvev" if key[0] == "cv" else "m2ev", key[1])
              for key in pe_seq[48:]
              if (key[0] == "m2" and key[1] not in ID_TILES) or
              (key[0] == "cv" and key[1] % 2 == 1 and key[1] not in (1, 3))]
    for g in range(NG):        # after early m2 tiles (or mid-c=0 conv evicts)
        anchor = ("m2ev", 2 + 2 * g) if not BISECT_NO_ILV else ("cvev", 13 + 4 * g)
        pos = stream.index(anchor) + 1
        stream[pos:pos] = [("diffp", g, 1, t) for t in range(T)]
    dve_ops += stream
    pool_ops = [("halo", i) for i in range(9)]   # 0..3 h1p, 4..7 gd, 8 gd tail

    ev_idx = {}   # op key -> (engine, 1-based index)
    for eng, ops in (("A", act_ops), ("P", pool_ops), ("D", dve_ops)):
        for pos, key in enumerate(ops):
            ev_idx[key] = (eng, pos + 1)

    def chunk_jc(n):
        return M1_CHUNKS[n]

    def m1_main_chunk(g, c, tc):
        j = c * 4 + min(tc + 1, 3)
        return j * 6 + (3 + g)       # main group g sits at M_ORDER index 3+g

    from contextlib import ExitStack
    _sems = ExitStack()
    xts = [_sems.enter_context(nc.semaphore(f"s_xt{i}")) for i in range(8)]
    xks = [_sems.enter_context(nc.semaphore(f"s_xk{i}")) for i in range(8)]
    ots = [_sems.enter_context(nc.semaphore(f"s_ot{i}")) for i in range(8)]
    with (
        _sems,
        nc.Block() as block,
        nc.semaphore("s_pe") as s_pe,
        nc.semaphore("s_evA") as s_evA,
        nc.semaphore("s_evP") as s_evP,
        nc.semaphore("s_evD") as s_evD,
        nc.semaphore("s_w1") as s_w1,
        nc.semaphore("s_w2") as s_w2,
        nc.semaphore("s_dgM") as s_dgM,
        nc.semaphore("s_dgO") as s_dgO,
        nc.semaphore("s_ld") as s_ld,
        nc.semaphore("s_z") as s_z,
        nc.semaphore("s_xc") as s_xc,
        nc.semaphore("s_ey") as s_ey,
    ):
        SEM = {"A": s_evA, "P": s_evP, "D": s_evD}

        def wait_ev(h, key):
            e, t = ev_idx[key]
            h.wait_ge(SEM[e], t)

        # ================= SP: all DMA =================
        @block.sync
        def _(sync):
            sync.dma_start(out=w1_sb[:], in_=w1c[:]).then_inc(s_w1, 16)
            for j in (0, 1, 2, 3):
                sync.dma_start(out=xT_sb[:, j * 2352:(j + 1) * 2352],
                               in_=xT[:, j * 2352:(j + 1) * 2352]).then_inc(xts[j], 16)
            sync.dma_start(out=zsb[:],
                           in_=bass.AP(zeros, 0, [[0, 128], [1, 960]])).then_inc(s_xc, 16)
            sync.dma_start(out=b1_sb[:], in_=b1c[:]).then_inc(s_ld, 16)
            sync.dma_start(out=cb_sb[:], in_=cbc[:]).then_inc(s_ld, 16)
            sync.dma_start(out=eye_sb[:], in_=eyec[:]).then_inc(s_ey, 16)
            for j in (4, 5, 6, 7):
                sync.dma_start(out=xT_sb[:, j * 2352:(j + 1) * 2352],
                               in_=xT[:, j * 2352:(j + 1) * 2352]).then_inc(xts[j], 16)
            # zero-fill h1p t-halo/guard runs (7x512)
            sync.dma_start(
                out=bass.AP(h1p, 0, [[H1_EXT, 128], [2560, 7], [1, 512]]),
                in_=bass.AP(zeros, 0, [[0, 128], [512, 7], [1, 512]]),
            ).then_inc(s_z, 16)
            o = diag_off(1, 0, 0)
            sync.dma_start(out=diag_sb[:, o:o + NG * NPR_MAIN * 256],
                           in_=diag[:, o:o + NG * NPR_MAIN * 256]).then_inc(s_dgM, 16)
            o = diag_off(0, 0, 0)
            sync.dma_start(out=diag_sb[:, o:o + NG * NPR_OFF * 256],
                           in_=diag[:, o:o + NG * NPR_OFF * 256]).then_inc(s_dgO, 16)
            sync.dma_start(out=w2_sb[:], in_=w2c[:]).then_inc(s_w2, 16)
            sync.dma_start(out=out[NPIX:OUT_ROWS, :], in_=xcls[:]).then_inc(s_xc, 16)
            for pj in range(12):     # all xtok pair-loads up front
                j = 2 * pj
                sync.dma_start(
                    out=xtk[:, j * C:(j + 2) * C].rearrange("p (b c) -> p b c", b=2),
                    in_=xtok[j * 128:(j + 2) * 128, :].rearrange("(b r) c -> r b c", b=2),
                ).then_inc(xks[pj % 8], 16)
            sync.dma_start(out=xtk[:64, bass.ts(24, C)],
                           in_=xtok[24 * 128:NPIX, :]).then_inc(xks[12 % 8], 16)
            ev = [0]
            def store_pair(i):
                wait_ev(sync, FIN[i])
                wait_ev(sync, FIN[i + 1])
                sync.dma_start(
                    out=out[i * 128:(i + 2) * 128, :].rearrange("(b r) c -> r b c", b=2),
                    in_=ost[:, (i % 8) * C:((i % 8) + 2) * C].rearrange("p (b c) -> p b c", b=2),
                ).then_inc(ots[ev[0] % 8], 16)
                ev[0] += 1
            for p in range(10):
                store_pair(2 * p)
            def store_one(i, rows=128):
                wait_ev(sync, FIN[i])
                sync.dma_start(out=out[i * 128:i * 128 + rows, :],
                               in_=ost[:rows, bass.ts(i % 8, C)]).then_inc(ots[ev[0] % 8], 16)
                ev[0] += 1
            store_one(20)
            store_one(21)
            store_one(22)
            store_one(23)
            store_one(24, rows=64)

        # ================= PE =================
        @block.tensor
        def _(tensor):
            for _w in range(WARMUP_N):
                tensor.matmul(ps[:, 3584:3648],
                              bass.AP(xT_sb, 0, [[XT_EXT, 128], [64, 2], [1, 128]]),
                              bass.AP(xT_sb, 0, [[XT_EXT, 128], [64, 2], [1, 64]]),
                              perf_mode=DR, start=True, stop=True,
                              skip_group_check=True)
            tensor.wait_ge(s_w1, 16)
            # ---- matmul1 (DR, 3 pair-passes), banks 0..7 rotating ----
            for n, (j, mi, m) in enumerate(M1_CHUNKS):
                bank = M1_BANK[n]
                if mi == 0:
                    tensor.wait_ge(xts[j], 16)
                if n in M1_PREV:
                    wait_ev(tensor, ("m1ev", M1_PREV[n]))
                pv = ps[:, bank * 512: bank * 512 + M1_CH]
                for pr in range(KC // 2):
                    lhsT = bass.AP(w1_sb, pr * 2 * C + m * 128,
                                   [[W_EXT, 128], [C, 2], [1, 128]])
                    rhs = bass.AP(xT_sb, j * 2352 + pr * 2 * M1_CH,
                                  [[XT_EXT, 128], [M1_CH, 2], [1, M1_CH]])
                    mm = tensor.matmul(pv, lhsT, rhs, perf_mode=DR,
                                       start=(pr == 0), stop=(pr == KC // 2 - 1))
                mm.then_inc(s_pe, 1)
            # ---- conv chunks with m2 tiles interleaved ----
            tensor.wait_ge(s_z, 16)
            tensor.wait_ge(s_w2, 16)
            for key in pe_seq[48:]:
                if key[0] == "cv":
                    k = key[1]
                    br, g, c, tc = CONV_CHUNKS[k]
                    bank = 4 + k % 4
                    if k == 0:
                        tensor.wait_ge(s_dgM, 16)
                        wait_ev(tensor, ("halo", 8))   # all halo fills done
                    if k == 3:
                        tensor.wait_ge(s_dgO, 16)
                    if k >= 4:
                        wait_ev(tensor, ("cvev", k - 4))
                    else:
                        wait_ev(tensor, ("m1ev", M1_LAST[4 + k]))
                    if br:
                        wait_ev(tensor, ("m1ev", m1_main_chunk(g, c, tc)))
                    else:
                        wait_ev(tensor, ("diffp", g, c, 2 * tc + 1))
                    pairs = MAIN_PAIRS if br else OFF_PAIRS
                    for ip, (tA, tB) in enumerate(pairs):
                        dtA, dhA, dwA = tA
                        if br:
                            offA = (h1_plane(g, c, 2 * tc + 1 + dtA)
                                    + (1 + dhA) * 16 + (1 + dwA))
                            buf, ext = h1p, H1_EXT
                        else:
                            offA = (g_plane(g, c, 2 * tc)
                                    + (1 + dhA) * 16 + (1 + dwA))
                            buf, ext = gd, GD_EXT
                        if tB is None:
                            sstep = 16
                        else:
                            dtB, dhB, dwB = tB
                            sstep = (dtB - dtA) * 256 + (dhB - dhA) * 16
                        lhsT = bass.AP(diag_sb, diag_off(br, g, ip),
                                       [[DG_EXT, 128], [128, 2], [1, 128]])
                        for pl in range(2):
                            rhs = bass.AP(buf, offA + pl * 256,
                                          [[ext, 128], [sstep, 2], [16, 14], [1, 14]])
                            pv = ps[:, bank * 512 + pl * 196:
                                    bank * 512 + pl * 196 + 196]
                            mm = tensor.matmul(pv, lhsT, rhs, perf_mode=DR,
                                               start=(ip == 0),
                                               stop=(ip == len(pairs) - 1),
                                               skip_group_check=True)
                    mm.then_inc(s_pe, 1)
                else:
                    i = key[1]
                    rows = min(128, NPIX - i * 128)
                    so = SLOT_OF[i]
                    if i in (21, 22):
                        pass                      # conv banks; coverage-waited
                    elif i == 23:
                        wait_ev(tensor, FIN[20])  # slot-0 previous user
                    elif i == 24:
                        wait_ev(tensor, FIN[19])  # slot-768 previous user
                    elif i >= 2:
                        wait_ev(tensor, FIN[i - 2])
                    else:
                        for b in range(4):
                            wait_ev(tensor, ("m1ev", M1_LAST[b]))
                    if i in ID_TILES:
                        tensor.wait_ge(s_ey, 16)
                        tensor.wait_ge(xks[(i // 2) % 8], 16 * (i // 2 // 8 + 1))
                    # cvo coverage: conv evicts of all chunks < K
                    p_hi = min(128 * (i + 1), NPIX) - 1
                    blk = (p_hi // NPIX_CLIP) * 4 + (p_hi % NPIX_CLIP) // 392
                    Kc = 6 * (blk + 1)
                    for eng, ops in (("A", act_ops), ("D", dve_ops)):
                        last = None
                        for pos, kk in enumerate(ops):
                            if kk[0] == "cvev" and kk[1] < Kc:
                                last = pos + 1
                        if last is not None:
                            tensor.wait_ge(SEM[eng], last)
                    ident = i in ID_TILES
                    for seg in range(3):      # one accumulation group at a time
                        for pr in range(KC // 2):
                            lhsT = bass.AP(cvo, (pr * 2) * NPIX + i * 128,
                                           [[CV_EXT, 128], [NPIX, 2], [1, rows]])
                            mm1 = tensor.matmul(
                                ps[:rows, so + seg * 256: so + seg * 256 + 256],
                                lhsT,
                                bass.AP(w2_sb, pr * 2 * C + seg * 256,
                                        [[W_EXT, 128], [C, 2], [1, 256]]),
                                perf_mode=DR,
                                start=(pr == 0),
                                stop=(pr == KC // 2 - 1 and not ident),
                                skip_group_check=True)
                        if ident:     # psum += 4096*x (bf16 identity matmul)
                            mm1 = tensor.matmul(
                                ps[:rows, so + seg * 256: so + seg * 256 + 256],
                                bass.AP(eye_sb, 0, [[128, 128], [1, rows]]),
                                bass.AP(xtk, i * C + seg * 256,
                                        [[NTIL2 * C, 128], [1, 256]]),
                                start=False, stop=True,
                                skip_group_check=True)
                    mm1.then_inc(s_pe, 1)

        # ---------- shared evict-op helpers ----------
        def m1_src(n):
            return bass.AP(ps, M1_BANK[n] * 512,
                           [[PS_EXT, 128], [196, 2], [14, 14], [1, 14]])

        def m1_dst(n):
            j, mi, m = chunk_jc(n)
            c, t = divmod(2 * j, T)
            if m < 3:
                return bass.AP(h1p, h1_plane(m, c, t + 1) + 17,
                               [[H1_EXT, 128], [256, 2], [16, 14], [1, 14]])
            return bass.AP(gp, g_plane(m - 3, c, t) + 17,
                           [[GP_EXT, 128], [256, 2], [16, 14], [1, 14]])

        def cv_src(k):
            return bass.AP(ps, (4 + k % 4) * 512, [[PS_EXT, 128], [1, 392]])

        def cv_dst(k):
            br, g, c, tc = CONV_CHUNKS[k]
            grp = g if br else 3 + g
            off = grp * NPIX + c * NPIX_CLIP + 2 * tc * 196
            return cvo[:, off: off + 392]

        # ================= ACT =================
        @block.scalar
        def _(scalar):
            scalar.wait_ge(s_ld, 32)
            for key in act_ops:
                scalar.wait_ge(s_pe, pe_ord[(key[0][:2], key[1])])
                if key[0] == "m1ev":
                    n = key[1]
                    m = chunk_jc(n)[2]
                    bias = b1_sb[:, m:m + 1] if m < 3 else 0.0
                    scalar.activation(m1_dst(n), m1_src(n), AFT.Identity,
                                      bias=bias).then_inc(s_evA, 1)
                elif key[0] == "cvev":
                    k = key[1]
                    br, g, c, tc = CONV_CHUNKS[k]
                    grp = g if br else 3 + g
                    scalar.activation(cv_dst(k), cv_src(k), AFT.Identity,
                                      bias=cb_sb[:, grp:grp + 1]).then_inc(s_evA, 1)
                else:                     # m2act: ost = psum * 1/4096 (x inside)
                    i = key[1]
                    rows = min(128, NPIX - i * 128)
                    if i >= 8:
                        e = (i - 8) // 2
                        scalar.wait_ge(ots[e % 8], 16 * (e // 8 + 1))
                    so = SLOT_OF[i]
                    scalar.activation(
                        ost[:rows, (i % 8) * C:(i % 8) * C + C],
                        ps[:rows, so: so + 768],
                        AFT.Identity, scale=1.0 / (CVS ** 3)).then_inc(s_evA, 1)

        # ================= Pool (gpsimd): halo zero-fills =================
        @block.gpsimd
        def _(pool):
            # memsets are async on Pool: every region must be byte-disjoint
            # from the others and from the h1p t-halo zero-DMA (so h1p
            # patterns cover only the 8 real planes per block; columns cover
            # only rows 1..14)
            pool.wait_ge(s_xc, 16)        # zsb ready (xcls inc comes later)
            for key in pool_ops:
                hi = key[1]
                if hi == 8:
                    pool.tensor_copy(gd[:, GPAD:GPAD + GUARD],
                                     zsb[:, 0:GUARD]).then_inc(s_evP, 1)
                    continue
                if hi // 4 == 0:
                    bb = GUARD + 256      # first real plane of block 0
                    off, dims = (
                        (bb, [[2560, 6], [256, 8], [1, 16]]),          # row 0
                        (bb + 240, [[2560, 6], [256, 8], [1, 16]]),    # row 15
                        (bb + 16, [[2560, 6], [256, 8], [16, 14]]),    # col 0
                        (bb + 31, [[2560, 6], [256, 8], [16, 14]]),    # col 15
                    )[hi % 4]
                    buf, ext = h1p, H1_EXT
                else:
                    off, dims = (
                        (0, [[256, 48], [1, 16]]),             # row 0
                        (240, [[256, 48], [1, 16]]),           # row 15
                        (16, [[256, 48], [16, 14]]),           # col 0
                        (31, [[256, 48], [16, 14]]),           # col 15
                    )[hi % 4]
                    buf, ext = gd, GD_EXT
                cnt = dims[-2][1] * dims[-1][1] * (dims[0][1] if len(dims) == 3 else 1)
                zdims = ([[dims[-1][1] * dims[-2][1], dims[0][1]]] if len(dims) == 3 else []) \
                    + [[dims[-1][1], dims[-2][1]], [1, dims[-1][1]]]
                pool.tensor_copy(bass.AP(buf, off, [[ext, 128]] + dims),
                                 bass.AP(zsb, 0, [[960, 128]] + zdims)
                                 ).then_inc(s_evP, 1)

        # ================= DVE =================
        @block.vector
        def _(vector):
            vector.wait_ge(s_ld, 32)
            for key in dve_ops:
                if key[0] == "m1ev":
                    n = key[1]
                    vector.wait_ge(s_pe, pe_ord[("m1", n)])
                    m = chunk_jc(n)[2]
                    bias = b1_sb[:, m:m + 1] if m < 3 else 0.0
                    vector.tensor_scalar(m1_dst(n), m1_src(n), bias, None,
                                         op0=AOT.add).then_inc(s_evD, 1)
                elif key[0] == "cvev":
                    k = key[1]
                    vector.wait_ge(s_pe, pe_ord[("cv", k)])
                    br, g, c, tc = CONV_CHUNKS[k]
                    grp = g if br else 3 + g
                    vector.tensor_scalar(cv_dst(k), cv_src(k),
                                         cb_sb[:, grp:grp + 1], None,
                                         op0=AOT.add).then_inc(s_evD, 1)
                elif key[0] == "diffp":
                    _, g, c, t = key
                    if t == 0:
                        Q = (c * 4 + 3) * 6 + g
                        cnt = sum(1 for kk in act_ops
                                  if kk[0] == "m1ev" and kk[1] <= Q)
                        vector.wait_ge(s_evA, cnt)
                    ob1 = b1_sb[:, 3 + g:4 + g]
                    iv1 = [[16, 14], [1, 14]]
                    a = g_plane(g, c, t) + 17
                    if t == 0:
                        vector.tensor_scalar(
                            bass.AP(gd, a, [[GD_EXT, 128]] + iv1),
                            bass.AP(gp, a, [[GP_EXT, 128]] + iv1), 0.0, ob1,
                            op0=AOT.mult, op1=AOT.add).then_inc(s_evD, 1)
                    else:
                        b = g_plane(g, c, t - 1) + 17
                        vector.scalar_tensor_tensor(
                            bass.AP(gd, a, [[GD_EXT, 128]] + iv1),
                            bass.AP(gp, a, [[GP_EXT, 128]] + iv1), ob1,
                            bass.AP(gp, b, [[GP_EXT, 128]] + iv1),
                            op0=AOT.add, op1=AOT.subtract).then_inc(s_evD, 1)
                else:                       # m2ev: STT evict + residual
                    i = key[1]
                    rows = min(128, NPIX - i * 128)
                    vector.wait_ge(s_pe, pe_ord[("m2", i)])
                    vector.wait_ge(xks[(i // 2) % 8], 16 * (i // 2 // 8 + 1))
                    if i >= 8:
                        e = (i - 8) // 2
                        vector.wait_ge(ots[e % 8], 16 * (e // 8 + 1))
                    so = SLOT_OF[i]
                    vector.scalar_tensor_tensor(
                        ost[:rows, (i % 8) * C:(i % 8) * C + C],
                        ps[:rows, so: so + 768],
                        1.0 / (CVS ** 3),
                        xtk[:rows, i * C:i * C + C],
                        op0=AOT.mult, op1=AOT.add).then_inc(s_evD, 1)

    return nc


# ---------------- host side ----------------
_NC_CACHE = {}


def _get_nc():
    if "nc" not in _NC_CACHE:
        _NC_CACHE["nc"] = build()
    return _NC_CACHE["nc"]


def _dr_pack(W):
    """[768(k), M] -> per-partition DR layout [128(ki), pair, s, M] flattened."""
    M = W.shape[1]
    out = np.zeros((128, KC // 2, 2, M), np.float32)
    for pr in range(KC // 2):
        for s in range(2):
            out[:, pr, s, :] = W[pr * 256 + s * 128: pr * 256 + (s + 1) * 128, :]
    return out.reshape(128, KC // 2 * 2 * M)


def _prep_weights(w1, b1, cw, cb, w2, b2, ow1, ob1, ocw, ocb, ow2, ob2):
    w1c = _dr_pack(np.hstack([w1, ow1]) * CVS).astype(F8NP)
    w2c = _dr_pack(np.vstack([w2, ow2]) * CVS).astype(F8NP)
    # diag tiles, (branch, group)-major: [128(ki), br/g/ip, s, 128(m)]
    diag = np.zeros((128, NPR_TOT, 2, 128), np.float32)
    eye = np.eye(128, dtype=bool)

    def tapw(w_, tp, main):
        dt, dh, dw = tp
        if main:
            return w_[:, 0, dt + 1, dh + 1, dw + 1]
        return w_[:, 0, 0, dh + 1, dw + 1]

    for main, (pairs, w_) in ((True, (MAIN_PAIRS, cw)), (False, (OFF_PAIRS, ocw))):
        for ip, (tA, tB) in enumerate(pairs):
            for g in range(NG):
                if main:
                    pi = g * NPR_MAIN + ip
                else:
                    pi = NG * NPR_MAIN + g * NPR_OFF + ip
                vA = tapw(w_, tA, main) * CVS
                diag[:, pi, 0, :][eye] = vA[g * 128:(g + 1) * 128]
                if tB is not None:
                    vB = tapw(w_, tB, main) * CVS
                    diag[:, pi, 1, :][eye] = vB[g * 128:(g + 1) * 128]
    b1cv = np.ascontiguousarray(
        (np.concatenate([b1, ob1]) * CVS).reshape(KC, 128).T).astype(np.float32)
    cbcv = np.ascontiguousarray(
        (np.concatenate([cb, ocb]) * CVS * CVS).reshape(KC, 128).T).astype(np.float32)
    bias2 = (b2 + ob2).astype(np.float32)
    return dict(w1c=w1c, w2c=w2c,
                diag=diag.reshape(128, NPR_TOT * 2 * 128).astype(F8NP),
                b1c=b1cv, cbc=cbcv,
                zeros=np.zeros((1, 3584), F8NP),
                eyec=(np.eye(128, dtype=np.float32) * CVS ** 3).astype(BF)), bias2


def _pack_xT(xpat):
    """[NPIX, C] f32 -> slice-major [128, j(8) k(6) 392] fp8."""
    A = np.ascontiguousarray(xpat.T).astype(F8NP)           # [768, 3136]
    A = A.reshape(KC, 128, 8, M1_CH)                        # [k, p, j, col]
    return np.ascontiguousarray(
        A.transpose(1, 2, 0, 3).reshape(128, KC * NPIX))    # [p, j, k, col]


def kernel(**inputs):
    x = np.asarray(inputs["x"], dtype=np.float32)
    Tv = int(np.asarray(inputs["T"]))
    assert Tv == T and x.shape == (128, 197, C)
    wd, bias2 = _prep_weights(
        *[np.asarray(inputs[k], dtype=np.float32) for k in
          ("w1", "b1", "cw", "cb", "w2", "b2", "ow1", "ob1", "ocw", "ocb", "ow2", "ob2")])

    in_maps = []
    for core in range(8):
        xs = x[core * 16:(core + 1) * 16]
        xpat = np.ascontiguousarray(xs[:, 1:, :]).reshape(NPIX, C)
        m = dict(wd)
        m["xT"] = _pack_xT(xpat)
        m["xtok"] = (xpat + bias2).astype(BF)
        m["xcls"] = np.ascontiguousarray(xs[:, 0, :]).astype(BF)
        in_maps.append(m)

    nc = _get_nc()
    res = run_bass_kernel_spmd(nc, in_maps, core_ids=list(range(8)))

    full = np.empty((128, 197, C), np.float32)
    for core in range(8):
        o = np.asarray(res.results[core]["out"], dtype=np.float32)
        full[core * 16:(core + 1) * 16, 0, :] = o[NPIX:NPIX + 16]
        full[core * 16:(core + 1) * 16, 1:, :] = o[:NPIX].reshape(16, 196, C)
    return full
